# revision 1
# baseline (speedup 1.0000x reference)
"""Trainium2 Bass kernel for nn_Encoder_39213051412927 (gnn_message_passing).

8-core SPMD, edge-parallel by destination node. Nodes are globally
degree-balanced into 40 bins (8 cores x 5 supertiles of 512 slots) so
every (core, supertile) owns ~1500 edges; edges live on the core that
owns their destination. Per step: one indirect-DMA gather per supertile
pulls all source rows (bf16) from the all-gathered node table, the
tensor engine computes per-edge ew tiles (bf16, o-major), the scalar
engine drains PSUM->SBUF, DVE multiplies by the gathered features
(free-dim broadcast) and tree-folds the i dimension with Pool taking
the narrow levels, and one-hot matmuls scatter messages into the
destination supertile with PSUM accumulation. Node tables are
all-gathered in bf16; collectives are ordered by explicit deps (no
all-engine barriers) so they overlap the edge-MLP compute.
"""

import sys

sys.path.insert(0, "/opt/trn_rl_repo")

import numpy as np
import ml_dtypes

import concourse.bass as bass
import concourse.tile as tile
from concourse import bacc, mybir
from concourse.bass_utils import run_bass_kernel_spmd
from concourse.masks import make_identity
from concourse.tile_rust import add_dep_helper

F32 = mybir.dt.float32
FP8 = mybir.dt.float8e4
BF16 = mybir.dt.bfloat16
I32 = mybir.dt.int32
bfloat16 = ml_dtypes.bfloat16

N = 20000
E = 60000
D = 64
HID = 768
EA = 85  # edge_attr dim = 21 + 64
NCORES = 8
ST_N = 512  # node slots per supertile
NST = 5  # supertiles per core
NDEV = NST * ST_N  # 2560 node slots per core
NFULL = NCORES * NDEV  # 20480
RELU = mybir.ActivationFunctionType.Relu
COPY = mybir.ActivationFunctionType.Copy
ADD = mybir.AluOpType.add
MULT = mybir.AluOpType.mult


def _prep(x, edge_index, edge_attr, inv_deg):
    """Host-side sharding. Returns per-core input maps (w/o weights) + C."""
    src = edge_index[0].astype(np.int64)
    dst = edge_index[1].astype(np.int64)
    deg = np.bincount(dst, minlength=N).astype(np.int64)

    # Global degree-balanced binning: 40 bins of <=512 nodes, greedily
    # assign highest-degree nodes to the least-loaded bin with space.
    NB = NCORES * NST
    order = np.argsort(-deg, kind="stable")
    load = np.zeros(NB, np.int64)
    cnt = np.zeros(NB, np.int64)
    g2dev = np.empty(N, np.int64)  # global node id -> slot in outfull
    dev2glob = np.full((NCORES, NDEV), -1, np.int64)
    BIG = 1 << 60
    for g in order:
        masked = np.where(cnt < ST_N, load, BIG)
        b = int(np.argmin(masked))
        c, st = b // NST, b % NST
        p = st * ST_N + cnt[b]
        cnt[b] += 1
        load[b] += deg[g]
        g2dev[g] = c * NDEV + p
        dev2glob[c, p] = g

    # outfull layouts (per step): one AllGather per split of supertiles;
    # within a split the collective concatenates cores, so slot =
    # st0*NCORES*512 + c*(st1-st0)*512 + (st-st0)*512 + sl
    g_c = g2dev // NDEV
    g_st = (g2dev % NDEV) // ST_N
    g_sl = g2dev % ST_N

    def _layout(splits):
        out = np.empty_like(g2dev)
        for st0, st1 in splits:
            m = (g_st >= st0) & (g_st < st1)
            out[m] = (st0 * NCORES * ST_N
                      + g_c[m] * (st1 - st0) * ST_N
                      + (g_st[m] - st0) * ST_N + g_sl[m])
        return out

    g2dev_l = {s: _layout(AG_SPLITS[s]) for s in (0, 1)}

    # edges grouped by (core, supertile of dst)
    e_dev = g2dev[dst]
    e_core = e_dev // NDEV
    e_st = (e_dev % NDEV) // ST_N
    bucket_of = e_core * NST + e_st
    bucket_sizes = np.bincount(bucket_of, minlength=NB)
    C = max(1, int((bucket_sizes.max() + 127) // 128))
    EC = NST * C
    Ep = EC * 128

    eorder = np.argsort(bucket_of, kind="stable")
    per_core = []
    for c in range(NCORES):
        ea_t = np.zeros((Ep, EA), np.float32)
        srcdev0 = np.zeros(Ep, np.int64)
        srcdev1 = np.zeros(Ep, np.int64)
        dstrel = np.full(Ep, 4096.0, np.float32)  # pad: no one-hot match
        for j in range(NST):
            b = c * NST + j
            es = eorder[np.searchsorted(bucket_of[eorder], b):
                        np.searchsorted(bucket_of[eorder], b, side="right")]
            o = j * C * 128
            k = len(es)
            if k:
                ea_t[o : o + k] = edge_attr[es]
                srcdev0[o : o + k] = g2dev_l[0][src[es]]
                srcdev1[o : o + k] = g2dev_l[1][src[es]]
                dstrel[o : o + k] = ((e_dev[es] % NDEV) % ST_N).astype(
                    np.float32
                )

        # node-side arrays in device order
        xd = np.zeros((NDEV, HID), np.float32)
        invd = np.ones(NDEV, np.float32)
        real = dev2glob[c] >= 0
        xd[real] = x[dev2glob[c][real]]
        invd[real] = inv_deg[dev2glob[c][real]]

        # per-supertile interleave: one contiguous DMA per supertile loads
        # all 6 contraction blocks [128, 6*512]
        xTr = (
            np.ascontiguousarray(
                xd.T.reshape(6, 128, NST, ST_N).transpose(1, 2, 0, 3)
            )
            .reshape(128, 6 * NDEV)
            .astype(bfloat16)
        )
        per_core.append(
            {
                "xTr": xTr,
                "eaT": np.ascontiguousarray(ea_t.T).astype(bfloat16),
                "srcdev0": np.ascontiguousarray(
                    srcdev0.reshape(EC, 128).T
                ).astype(np.int32),
                "srcdev1": np.ascontiguousarray(
                    srcdev1.reshape(EC, 128).T
                ).astype(np.int32),
                # one-hot weighted by 1/deg of the destination slot:
                # the scatter then accumulates the mean directly
                "ohT": np.ascontiguousarray(
                    ((dstrel.reshape(EC, 128, 1)
                      == np.arange(ST_N)[None, None, :])
                     * invd.reshape(NST, ST_N)[
                         np.arange(EC) // C][:, None, :])
                    .transpose(1, 0, 2).reshape(128, EC * ST_N)
                ).astype(bfloat16),
            }
        )
    return per_core, dev2glob, C


def _weights_map(lin0_w, lin0_b, linh_w, linh_b, linhm_w, linhm_b,
                 en1_w, en1_b, en2_w, en2_b, conv_b):
    w2aug = np.concatenate([en2_w, en2_b[None, :]], axis=0)  # [65, 4096] (i,o)
    w2aug = (
        w2aug.reshape(65, D, D).transpose(0, 2, 1).reshape(65, D * D)
    )  # o-major
    lin0_wr = np.ascontiguousarray(
        lin0_w.reshape(6, 128, D).transpose(1, 0, 2).reshape(128, 6 * D)
    )
    col = lambda v: np.ascontiguousarray(v.reshape(-1, 1)).astype(np.float32)
    return {
        "lin0_wr": lin0_wr.astype(bfloat16),
        "lin0_b": col(lin0_b),
        "en1_w": en1_w.astype(bfloat16),
        "en1_b": col(en1_b),
        "w2aug": w2aug.astype(bfloat16),
        "linh_w": linh_w.astype(np.float32),
        "linh_b": col(linh_b),
        "linhm_w": linhm_w.astype(np.float32),
        "linhm_b": col(linhm_b),
        "conv_b": col(conv_b),
    }


_BUILD_CACHE = {}

# fold level -> engine ("v" = DVE, "p" = Pool); applied widest first.
# The scatter matmuls absorb the last fold levels via PSUM accumulation
# over r in 0..FOLD_W-1. Step 0 keeps the scatter narrow (PE also runs
# the ew matmuls there); step 1 widens it (PE is idle, DVE relieved).
FOLD_SCHED = {
    0: [("v", 32), ("v", 16), ("p", 8), ("p", 4), ("v", 2)],
    1: [("v", 32), ("v", 16), ("p", 8), ("p", 4)],
}
FOLD_W = {0: 2, 1: 4}

# node tables are all-gathered in contiguous supertile splits, per step:
# AG0 front-loads a small split (starts during P1); AG1 keeps the LAST
# split small (less exposure at the step boundary)
AG_SPLITS = {0: [(0, 1), (1, 5)], 1: [(0, 3), (3, 5)]}

# software-pipeline depth: ew/oh/gather of chunk q+K are emitted before
# the mult/fold/scatter of chunk q so the in-order engine sequencers
# have AG-independent work queued across collective waits
PIPE_K = 9


def _build(C):
    if C in _BUILD_CACHE:
        return _BUILD_CACHE[C]
    EC = NST * C
    Ep = EC * 128

    nc = bacc.Bacc("TRN2", target_bir_lowering=False, debug=False,
                   num_devices=NCORES)

    # ---- I/O ----
    d_xTr = nc.dram_tensor("xTr", [128, 6 * NDEV], BF16, kind="ExternalInput")
    d_eaT = nc.dram_tensor("eaT", [EA, Ep], BF16, kind="ExternalInput")
    d_src0 = nc.dram_tensor("srcdev0", [128, EC], I32, kind="ExternalInput")
    d_src1 = nc.dram_tensor("srcdev1", [128, EC], I32, kind="ExternalInput")
    d_ohT = nc.dram_tensor("ohT", [128, EC * ST_N], BF16,
                           kind="ExternalInput")
    d_lin0_wr = nc.dram_tensor("lin0_wr", [128, 6 * D], BF16, kind="ExternalInput")
    d_lin0_b = nc.dram_tensor("lin0_b", [D, 1], F32, kind="ExternalInput")
    d_en1_w = nc.dram_tensor("en1_w", [EA, D], BF16, kind="ExternalInput")
    d_en1_b = nc.dram_tensor("en1_b", [D, 1], F32, kind="ExternalInput")
    d_w2aug = nc.dram_tensor("w2aug", [65, D * D], BF16, kind="ExternalInput")
    d_linh_w = nc.dram_tensor("linh_w", [D, D], F32, kind="ExternalInput")
    d_linh_b = nc.dram_tensor("linh_b", [D, 1], F32, kind="ExternalInput")
    d_linhm_w = nc.dram_tensor("linhm_w", [2 * D, D], F32, kind="ExternalInput")
    d_linhm_b = nc.dram_tensor("linhm_b", [D, 1], F32, kind="ExternalInput")
    d_conv_b = nc.dram_tensor("conv_b", [D, 1], F32, kind="ExternalInput")
    d_y = nc.dram_tensor("y", [NDEV, D], F32, kind="ExternalOutput")

    # internal DRAM
    rows0 = nc.dram_tensor("rows0", [NDEV, D], FP8)
    rows1 = nc.dram_tensor("rows1", [NDEV, D], FP8)
    ewcache = nc.dram_tensor("ewcache", [128, EC * D * D], BF16)
    outfull0 = nc.dram_tensor("outfull0", [NFULL, D], FP8, addr_space="Shared")
    outfull1 = nc.dram_tensor("outfull1", [NFULL, D], FP8, addr_space="Shared")

    groups = [list(range(NCORES))]

    with tile.TileContext(nc, num_cores=NCORES) as tc:
        with (
            tc.tile_pool(name="wp", bufs=1) as wp,
            tc.tile_pool(name="state", bufs=1) as stp,
            tc.tile_pool(name="ewp", bufs=11) as ewp,
            tc.tile_pool(name="ohp", bufs=11) as ohp,
            tc.tile_pool(name="sgp", bufs=2) as sgp,
            tc.tile_pool(name="wk", bufs=2) as wk,
            tc.tile_pool(name="x1", bufs=2) as x1p,
            tc.tile_pool(name="ewps", bufs=2, space="PSUM") as ewps,
            tc.tile_pool(name="aggp", bufs=2, space="PSUM") as aggps,
            tc.tile_pool(name="mmp", bufs=1, space="PSUM") as mmps,
            tc.tile_pool(name="trp", bufs=1, space="PSUM") as trps,
        ):
            # ---- constants / weights ----
            def load(pool, shape, dt, dram, tag):
                t = pool.tile(shape, dt, tag=tag, name=tag)
                nc.sync.dma_start(t[:], dram[:, :])
                return t

            # only P1's weights load before P1 (HWDGE dispatch is in
            # program order; P1 feeds AG0 which gates everything)
            lin0_wr = load(wp, [128, 6 * D], BF16, d_lin0_wr, "lin0_wr")
            lin0_b = load(wp, [D, 1], F32, d_lin0_b, "lin0_b")

            ident = wp.tile([128, 128], F32, tag="ident", name="ident")
            make_identity(nc, ident[:])

            def store_rows(srcT, j, rows_dram, dt, tag):
                """transpose srcT[:, j*512:(j+1)*512] and store those 512
                rows with a single DMA via a [128, 256] staging tile."""
                stage = wk.tile([128, 4 * D], dt, tag=tag, name=tag)
                for t in range(ST_N // 128):
                    tt = j * (ST_N // 128) + t
                    tp = trps.tile([128, 64], F32, tag="tr", name="tr")
                    nc.tensor.transpose(
                        tp[:], srcT[:, tt * 128 : (tt + 1) * 128],
                        ident[0:64, 0:64],
                    )
                    nc.scalar.activation(
                        stage[:, t * D : (t + 1) * D], tp[:], COPY
                    )
                r_ap = rows_dram[:, :]
                out_ap = bass.AP(r_ap.tensor, j * ST_N * D,
                                 [[D, 128], [128 * D, 4], [1, D]])
                s_ap = stage[:, :]
                in_ap = bass.AP(s_ap.tensor, s_ap.offset,
                                [s_ap.ap[0], [D, 4], [1, D]])
                return nc.sync.dma_start(out_ap, in_ap)

            # ---- P1: out0T = relu(x @ lin0_w + b) -> rows0 (bf16) ----
            out0T = stp.tile([64, NDEV], F32, tag="out0T", name="out0T")
            rows0_stores = []
            ag0_list = []
            for j in range(NST):
                xt = x1p.tile([128, 6 * ST_N], BF16, tag="xt", name="xt")
                nc.sync.dma_start(
                    xt[:], d_xTr[:, j * 6 * ST_N : (j + 1) * 6 * ST_N]
                )
                ps = mmps.tile([64, ST_N], F32, tag="mm", name="mm")
                for k in range(6):
                    nc.tensor.matmul(
                        ps[:],
                        lin0_wr[:, k * D : (k + 1) * D],
                        xt[:, k * ST_N : (k + 1) * ST_N],
                        start=(k == 0),
                        stop=(k == 5),
                    )
                nc.scalar.activation(
                    out0T[:, j * ST_N : (j + 1) * ST_N], ps[:], RELU,
                    bias=lin0_b[:, :1],
                )
                # transpose + store this supertile's rows (bf16)
                rows0_stores.append(
                    store_rows(out0T, j, rows0, FP8, "stgb")
                )
                done = [sp for sp in AG_SPLITS[0] if sp[1] == j + 1]
                if done:
                    st0, st1 = done[0]
                    ag0_sp = nc.gpsimd.collective_compute(
                        "AllGather", mybir.AluOpType.bypass,
                        replica_groups=groups,
                        ins=[rows0[st0 * ST_N : st1 * ST_N, :]],
                        outs=[outfull0[st0 * NCORES * ST_N :
                                       st1 * NCORES * ST_N, :]],
                    )
                    for st_dma in rows0_stores:
                        add_dep_helper(ag0_sp.ins, st_dma.ins,
                                       reason="AG0 after rows0")
                    rows0_stores = []
                    ag0_list.append(ag0_sp)

            # remaining weights/tables (deferred so their DMA dispatch
            # doesn't delay P1)
            en1_w = load(wp, [EA, D], BF16, d_en1_w, "en1_w")
            en1_b = load(wp, [D, 1], F32, d_en1_b, "en1_b")
            w2aug = load(wp, [65, D * D], BF16, d_w2aug, "w2aug")
            linh_w = load(wp, [D, D], F32, d_linh_w, "linh_w")
            linh_b = load(wp, [D, 1], F32, d_linh_b, "linh_b")
            linhm_w = load(wp, [2 * D, D], F32, d_linhm_w, "linhm_w")
            linhm_b = load(wp, [D, 1], F32, d_linhm_b, "linhm_b")
            conv_b = load(wp, [D, 1], F32, d_conv_b, "conv_b")
            srcdev0 = load(wp, [128, EC], I32, d_src0, "srcdev0")
            srcdev1 = load(wp, [128, EC], I32, d_src1, "srcdev1")

            # ---- P2: h2aug (overlaps AG0) ----
            h2aug = stp.tile([65, Ep], BF16, tag="h2aug", name="h2aug")
            nc.vector.memset(h2aug[64:65, :], 1.0)
            eaT = stp.tile([EA, Ep], BF16, tag="eaT", name="eaT")
            nc.sync.dma_start(eaT[:], d_eaT[:, :])
            for q in range(Ep // ST_N):
                ps = mmps.tile([64, ST_N], F32, tag="mm", name="mm")
                nc.tensor.matmul(
                    ps[:], en1_w[:, :], eaT[:, q * ST_N : (q + 1) * ST_N],
                    start=True, stop=True,
                )
                nc.scalar.activation(
                    h2aug[0:64, q * ST_N : (q + 1) * ST_N], ps[:], RELU,
                    bias=en1_b[:, :1],
                )

            # ---- steps ----
            hT = out0T  # h0 = out0
            ag_insts = [ag0_list, None]
            ewc_writes = {}
            for s in range(2):
                src_tbl = srcdev0 if s == 0 else srcdev1
                outfull = outfull0 if s == 0 else outfull1
                catT = stp.tile([128, NDEV], F32, tag=f"cat{s}", name=f"cat{s}")
                outnT = stp.tile([64, NDEV], F32, tag=f"outn{s}",
                                 name=f"outn{s}")
                pending_stores = []
                ag1_list = []
                pend = {}

                def emit_front(ec):
                    """one-hot + gather + ew for chunk ec (AG-free except
                    the gather, which parks in the Pool queue). Step 0
                    computes ew and caches it to DRAM (before the in-place
                    multiply); step 1 reloads it via DMA instead of
                    recomputing, freeing PE and the scalar engine."""
                    oh = ohp.tile([128, ST_N], BF16, tag="oh", name="oh")
                    nc.sync.dma_start(
                        oh[:], d_ohT[:, ec * ST_N : (ec + 1) * ST_N]
                    )
                    sg8 = sgp.tile([128, 64], FP8, tag="sg8", name="sg8")
                    g = nc.gpsimd.indirect_dma_start(
                        out=sg8[:],
                        out_offset=None,
                        in_=outfull[:, :],
                        in_offset=bass.IndirectOffsetOnAxis(
                            ap=src_tbl[:, ec : ec + 1], axis=0
                        ),
                    )
                    for ag in ag_insts[s]:
                        add_dep_helper(g.ins, ag.ins, reason="gather after AG")
                    sg = sgp.tile([128, 64], BF16, tag="sg", name="sg")
                    if s == 0:
                        # Act paces step 0 (drains); convert on Pool there
                        nc.gpsimd.tensor_copy(sg[:], sg8[:])
                    else:
                        # Act is idle in step 1; free Pool for fold work
                        nc.scalar.activation(sg[:], sg8[:], COPY)
                    ew = ewp.tile([128, D * D], BF16, tag="ew", name="ew")
                    csl = slice(ec * D * D, (ec + 1) * D * D)
                    if s == 0:
                        for p in range(4):
                            eps = ewps.tile([128, 1024], F32, tag="ewps",
                                            name="ewps")
                            for h in range(2):
                                nc.tensor.matmul(
                                    eps[:, h * 512 : (h + 1) * 512],
                                    h2aug[:, ec * 128 : (ec + 1) * 128],
                                    w2aug[:, p * 1024 + h * 512 :
                                          p * 1024 + (h + 1) * 512],
                                    start=True, stop=True,
                                )
                            nc.scalar.activation(
                                ew[:, p * 1024 : (p + 1) * 1024], eps[:], COPY
                            )
                        ewc_writes[ec] = nc.sync.dma_start(
                            ewcache[:, csl], ew[:]
                        )
                    else:
                        ld = nc.sync.dma_start(ew[:], ewcache[:, csl])
                        add_dep_helper(ld.ins, ewc_writes[ec].ins,
                                       reason="ew reload after cache write")
                    pend[ec] = (ew, oh, sg)


                # h-branch for all supertiles: AG-independent, overlaps AG
                for st in range(NST):
                    sl_ = slice(st * ST_N, (st + 1) * ST_N)
                    psh = mmps.tile([64, ST_N], F32, tag="mm", name="mm")
                    nc.tensor.matmul(psh[:], linh_w[:, :], hT[:, sl_],
                                     start=True, stop=True)
                    nc.scalar.activation(catT[0:64, sl_], psh[:], RELU,
                                         bias=linh_b[:, :1])

                for ec0 in range(min(PIPE_K, EC)):
                    emit_front(ec0)

                def emit_scatter(agg, ew_ap, oh, first, last):
                    """one-hot scatter matmuls, PSUM-accumulating into agg"""
                    for sl in range(ST_N // 128):
                        for r in range(FOLD_W[s]):
                            lhsT = bass.AP(ew_ap.tensor, ew_ap.offset + r,
                                           [ew_ap.ap[0], [64, 64]])
                            nc.tensor.matmul(
                                agg[:, sl * 128 : (sl + 1) * 128],
                                lhsT,
                                oh[:, sl * 128 : (sl + 1) * 128],
                                start=(first and r == 0 and sl == 0),
                                stop=(last and r == FOLD_W[s] - 1
                                      and sl == ST_N // 128 - 1),
                            )

                for st in range(NST):
                    agg = aggps.tile([64, ST_N], F32, tag="agg", name="agg")
                    # scatter lags the fold by one chunk so PE's in-order
                    # stream never parks >4 matmuls on an unfinished fold
                    sc_prev = None
                    for q in range(C):
                        ec = st * C + q
                        if ec + PIPE_K < EC:
                            emit_front(ec + PIPE_K)
                        if sc_prev is not None:
                            emit_scatter(agg, *sc_prev)
                        ew, oh, sg = pend.pop(ec)
                        # multiply by gathered features (DVE, bf16 2x)
                        ew_ap = ew[:, :]
                        ew3 = bass.AP(ew_ap.tensor, ew_ap.offset,
                                      [ew_ap.ap[0], [64, 64], [1, 64]])
                        sg_ap = sg[:, :]
                        sg3 = bass.AP(sg_ap.tensor, sg_ap.offset,
                                      [sg_ap.ap[0], [0, 64], [1, 64]])
                        nc.vector.tensor_tensor(out=ew3, in0=ew3, in1=sg3,
                                                op=MULT)
                        # tree-fold i per FOLD_SCHED; in step 1 the w16
                        # level is o-split so Pool absorbs a quarter of it
                        for eng, w in FOLD_SCHED[s]:
                            splits = ([("v", 0, 48), ("p", 48, 64)]
                                      if s == 1 and w == 16
                                      else [(eng, 0, 64)])
                            for e2, o0, o1 in splits:
                                lo = bass.AP(ew_ap.tensor,
                                             ew_ap.offset + o0 * 64,
                                             [ew_ap.ap[0], [64, o1 - o0],
                                              [1, w]])
                                hi = bass.AP(ew_ap.tensor,
                                             ew_ap.offset + o0 * 64 + w,
                                             [ew_ap.ap[0], [64, o1 - o0],
                                              [1, w]])
                                e_ = (nc.vector if e2 == "v"
                                      else nc.gpsimd)
                                e_.tensor_tensor(out=lo, in0=lo, in1=hi,
                                                 op=ADD)
                        sc_prev = (ew_ap, oh, q == 0, False)
                    emit_scatter(agg, sc_prev[0], sc_prev[1],
                                 first=(C == 1), last=True)
                    # supertile epilogue (h-branch already done above);
                    # agg already holds the mean (inv-deg folded into ohT)
                    sl_ = slice(st * ST_N, (st + 1) * ST_N)
                    nc.scalar.activation(catT[64:128, sl_], agg[:], RELU,
                                         bias=conv_b[:, :1])
                    psm = mmps.tile([64, ST_N], F32, tag="mm", name="mm")
                    nc.tensor.matmul(psm[:], linhm_w[:, :], catT[:, sl_],
                                     start=True, stop=True)
                    nc.scalar.activation(outnT[:, sl_], psm[:], RELU,
                                         bias=linhm_b[:, :1])
                    nc.vector.tensor_tensor(out=outnT[:, sl_],
                                            in0=outnT[:, sl_],
                                            in1=hT[:, sl_], op=ADD)
                    # transpose + store out rows (bf16 rows1 / f32 y)
                    if s == 0:
                        st_dma = store_rows(outnT, st, rows1, FP8, "stgb")
                    else:
                        st_dma = store_rows(outnT, st, d_y, F32, "stgf")
                    if s == 0:
                        pending_stores.append(st_dma)
                        done = [sp for sp in AG_SPLITS[1] if sp[1] == st + 1]
                        if done:
                            st0, st1 = done[0]
                            # partial AllGather of finished supertiles,
                            # overlaps the remaining step-0 work
                            ag1_sp = nc.gpsimd.collective_compute(
                                "AllGather", mybir.AluOpType.bypass,
                                replica_groups=groups,
                                ins=[rows1[st0 * ST_N : st1 * ST_N, :]],
                                outs=[outfull1[st0 * NCORES * ST_N :
                                               st1 * NCORES * ST_N, :]],
                            )
                            for st_dma in pending_stores:
                                add_dep_helper(ag1_sp.ins, st_dma.ins,
                                               reason="AG1 after rows1")
                            pending_stores = []
                            ag1_list.append(ag1_sp)
                # step tail
                hT = catT[0:64, :]
                if s == 0:
                    ag_insts[1] = ag1_list

    nc.finalize()
    _BUILD_CACHE[C] = nc
    return nc


def kernel(x, edge_index, edge_attr, lin0_w, lin0_b, linh_w, linh_b,
           linhm_w, linhm_b, en1_w, en1_b, en2_w, en2_b, conv_b):
    x = np.asarray(x, np.float32)
    edge_index = np.asarray(edge_index)
    edge_attr = np.asarray(edge_attr, np.float32)

    dst = edge_index[1].astype(np.int64)
    deg = np.bincount(dst, minlength=N).astype(np.float32)
    inv_deg = 1.0 / np.maximum(deg, 1.0)

    per_core, dev2glob, C = _prep(x, edge_index, edge_attr, inv_deg)
    wmap = _weights_map(
        np.asarray(lin0_w, np.float32), np.asarray(lin0_b, np.float32),
        np.asarray(linh_w, np.float32), np.asarray(linh_b, np.float32),
        np.asarray(linhm_w, np.float32), np.asarray(linhm_b, np.float32),
        np.asarray(en1_w, np.float32), np.asarray(en1_b, np.float32),
        np.asarray(en2_w, np.float32), np.asarray(en2_b, np.float32),
        np.asarray(conv_b, np.float32),
    )
    nc = _build(C)
    in_maps = [dict(per_core[c], **wmap) for c in range(NCORES)]
    res = run_bass_kernel_spmd(nc, in_maps, list(range(NCORES)))
    global LAST_RES
    LAST_RES = res

    out = np.zeros((N, D), np.float32)
    for c in range(NCORES):
        real = dev2glob[c] >= 0
        out[dev2glob[c][real]] = res.results[c]["y"][real]
    return out

